# revision 51
# baseline (speedup 1.0000x reference)
"""Trainium2 Bass kernel for nn_Attention (dense_transformer).

B=8 batches -> pure data-parallel across 8 NeuronCores (one batch per core,
weights replicated, no collectives).

Per-core dataflow (all matmuls keep seq on the free axis, no on-chip
transposes anywhere):
  - host packs dec.T, mem.T, Wq.T/Wk.T/Wv.T, Wf_aug.T into big f32 DRAM
    tensors; DMAs are chunked so the first projection starts ~2.5us in
  - projections produce Q.T/K.T (fp32r: 4x faster than fp32 on the PE at
    identical observed precision) and V in bf16 with a ones column per head
  - the gaussian bias makes attention banded: exp(-(k-q)^2/30) underflows
    for |k-q| >= 52, so each 128-row k-tile only touches a <=384-wide q
    window (22 of 64 blocks computed; the rest of the attn output stays in
    the runtime's pre-zeroed output buffer)
  - scores S.T[k,q] = K_h @ Q_h.T via fp32r matmuls, two heads packed in
    the PE array with tile_position row groups
  - softmax needs no max subtraction (scores bounded): ACT computes
    exp(0.125*s) straight out of PSUM into the U tile (bf16)
  - DVE multiplies in place by host-precomputed E.T = exp(-gauss)*(1-mask.T)
  - AV matmuls (bf16) contract k on partitions, accumulating per-element
    into a zero-initialized PSUM bank; the ones column lands the softmax
    denominator Z in psum row 64
  - zq = query_mask/Z broadcast to 128 partitions via a tiny PE matmul;
    normalization is a bf16 multiply split across DVE/GPSIMD
  - residual + bias fold into Wf_aug host-side (identity block + bias row);
    final linear is fp32r with cat.T = [dec.T; res.T; ones] stationary
  - LayerNorm via fused bn_stats/bn_aggr over the free axis, f32 out
  - attn written to DRAM as [h, k, q] bf16; host returns a transposed view

A post-pass splits multi-wait instructions into single-wait NoOps (the
walrus build here rejects >1 sync-wait per instruction).
"""
import os
import sys
sys.path.insert(0, "/opt/trn_rl_repo")
os.environ.setdefault("JAX_PLATFORMS", "axon,cpu")

import numpy as np
import ml_dtypes
from contextlib import ExitStack

import concourse.bass as bass
import concourse.tile as tile
from concourse import mybir
from concourse.bass_utils import run_bass_kernel_spmd
from concourse.tile import add_dep_helper

N_CORES = 8
B, S, D, H = 8, 1024, 512, 8
DH = D // H          # 64
HP = H // 2          # 4 head pairs
KT = S // 128        # 8 k tiles
QC = S // 512        # 2 q chunks of 512
LN_EPS = 1e-5

F32 = mybir.dt.float32
F32R = mybir.dt.float32r
BF16 = mybir.dt.bfloat16

# pk1 packed cols: decT (4x1024)
PK1_DEC = 0
PK1_F = 4096
# pk2 packed cols: memT (4x1024) | WkT (4x512) | WvT (4x512) | WqT (4x512)
PK2_MEM = 0
PK2_WK = 4096
PK2_WV = 6144
PK2_WQ = 8192
PK2_F = 10240
# pk3 packed cols: WfT_aug (8x512) | bf row (512)
PK3_WF = 0
PK3_BF = 4096
PK3_F = 4608
VW = DH + 1          # 65: V columns per head incl. ones column
# banding: exp(-(k-q)^2/gfac) underflows f32/bf16 for |k-q| >= 52, so each
# k-tile kt only interacts with q in [128kt-52, 128kt+180). Rounded to
# 128-blocks:
BLO = [max(0, (128 * kt - 52) // 128) for kt in range(KT)]
BHI = [min(KT - 1, (128 * kt + 179) // 128) for kt in range(KT)]
QLO = [128 * b for b in BLO]
QWIN = [128 * (BHI[kt] - BLO[kt] + 1) for kt in range(KT)]     # 256..384
UOFF = [0]
for kt in range(KT):
    UOFF.append(UOFF[-1] + 2 * QWIN[kt])
EPOFF = [0]
for kt in range(KT):
    EPOFF.append(EPOFF[-1] + QWIN[kt])
EP_F = EPOFF[-1]                                               # 2816
# kts covering each q-block, in order
BKTS = [[kt for kt in range(KT) if BLO[kt] <= b <= BHI[kt]] for b in range(KT)]


def _split_multi_waits(nc):
    """walrus in this container rejects >1 sync-wait per instruction.
    Hoist extra waits onto single-wait NoOps inserted just before."""
    import bass_rust
    cnt = 0
    for f in nc.m.functions:
        for blk in f.blocks:
            new_insts = []
            for inst in blk.instructions:
                si = getattr(inst, "sync_info", None)
                if si is not None and si.on_wait and len(si.on_wait) > 1:
                    waits = list(si.on_wait)
                    for w in waits[:-1]:
                        cnt += 1
                        nop = mybir.InstNoOp(name=f"W-split-{cnt}", ins=[], outs=[])
                        nop.engine = inst.engine
                        nop.sync_info = bass_rust.SyncInfo(on_wait=[w], on_update=[])
                        new_insts.append(nop)
                    inst.sync_info = bass_rust.SyncInfo(
                        on_wait=[waits[-1]], on_update=list(si.on_update))
                new_insts.append(inst)
            blk.instructions = new_insts
    return cnt


def build(split=True):
    nc = bass.Bass("TRN2", target_bir_lowering=False, debug=False,
                   num_devices=N_CORES)
    pk1 = nc.dram_tensor("pk1", [128, PK1_F], F32, kind="ExternalInput").ap()
    pk2 = nc.dram_tensor("pk2", [128, PK2_F], F32, kind="ExternalInput").ap()
    pk3 = nc.dram_tensor("pk3", [128, PK3_F], F32, kind="ExternalInput").ap()
    epd = nc.dram_tensor("ep", [128, EP_F], BF16, kind="ExternalInput").ap()
    qmd = nc.dram_tensor("qm", [1, S], F32, kind="ExternalInput").ap()
    gbd = nc.dram_tensor("gbr", [1, 2 * D + 128], F32, kind="ExternalInput").ap()
    out_ln = nc.dram_tensor("out_ln", [S, D], F32, kind="ExternalOutput").ap()
    attn_o = nc.dram_tensor("attn_o", [H, S, S], BF16, kind="ExternalOutput").ap()

    with tile.TileContext(nc) as tc, ExitStack() as ctx:
        per = ctx.enter_context(tc.tile_pool(name="per", bufs=1))
        work = ctx.enter_context(tc.tile_pool(name="work", bufs=2))
        uhp = ctx.enter_context(tc.tile_pool(name="uhp", bufs=1))
        psum = ctx.enter_context(tc.tile_pool(name="psum", bufs=1, space="PSUM"))

        # ---- persistent tiles / input DMAs -------------------------------
        qmt = per.tile([1, S], F32)
        nc.sync.dma_start(qmt[:], qmd[:])
        gbt = per.tile([1, 2 * D + 128], F32R)
        nc.sync.dma_start(gbt[:], gbd[:].bitcast(F32R))
        pk1t = per.tile([128, PK1_F], F32R)
        pk2t = per.tile([128, PK2_F], F32R, tag="pkshared")
        for di in range(4):
            for lo, wdt in ((PK2_WQ + di * 512, 512), (PK1_DEC + di * 1024, None),
                            (PK2_WK + di * 512, 512), (PK2_MEM + di * 1024, 1024)):
                if wdt is None:
                    nc.sync.dma_start(pk1t[:, lo:lo + 1024],
                                      pk1[:, lo:lo + 1024].bitcast(F32R))
                else:
                    nc.sync.dma_start(pk2t[:, lo:lo + wdt],
                                      pk2[:, lo:lo + wdt].bitcast(F32R))
        for di in range(4):
            nc.sync.dma_start(pk2t[:, PK2_WV + di * 512:PK2_WV + (di + 1) * 512],
                              pk2[:, PK2_WV + di * 512:PK2_WV + (di + 1) * 512].bitcast(F32R))
        ept = per.tile([128, EP_F], BF16)
        ep_dma = nc.sync.dma_start(ept[:], epd[:])

        qtT = per.tile([128, HP * S], F32R)     # Q.T packed, pair hp at cols hp*1024
        ktT = per.tile([128, HP * S], F32R)     # K.T packed
        vaug = per.tile([128, KT * H * VW], BF16)  # V + ones col, ktile kt at cols kt*520
        catr = per.tile([128, HP * S], F32R)    # res.T packed (cat rows 512..1023)
        onesb = per.tile([1, 128], BF16)        # bf16 ones row
        zrow = per.tile([1, VW], BF16)          # bf16 zeros row (psum init)
        gb = per.tile([128, D], F32)            # gamma broadcast
        bb = per.tile([128, D], F32)            # beta broadcast
        epst = per.tile([128, 1], F32)          # LN eps per-partition const

        nc.gpsimd.memset(vaug[:], 1.0)
        nc.vector.memset(epst[:], LN_EPS)
        nc.gpsimd.memset(onesb[:], 1.0)
        nc.gpsimd.memset(zrow[:], 0.0)

        _psn = [0]
        PS_BUFS = {"st": 4, "av": 2, "misc": 2}
        def ps(tag, p=128, n=512):
            _psn[0] += 1
            return psum.tile([p, n], F32, tag=tag, bufs=PS_BUFS[tag],
                             name=f"ps_{tag}_{_psn[0]}")

        # attn regions outside the gaussian band stay zero: ExternalOutput
        # buffers are pre-zeroed by the runtime, so nothing to write there.

        # ---- gamma/beta broadcast to 128 partitions ----------------------
        for i, dst in enumerate((gb, bb)):
            p = ps("misc")
            nc.tensor.matmul(p[:], gbt[0:1, 2 * D:2 * D + 128], gbt[0:1, i * D:(i + 1) * D],
                             start=True, stop=True)
            nc.scalar.copy(dst[:], p[:])

        # ---- projections: Q.T, K.T (f32r), V -> vaug (bf16) --------------
        def proj_qk(hp):
            for qc in range(QC):
                p = ps("misc")
                for di in range(4):
                    nc.tensor.matmul(
                        p[:],
                        pk2t[:, PK2_WQ + di * 512 + hp * 128:PK2_WQ + di * 512 + (hp + 1) * 128],
                        pk1t[:, PK1_DEC + di * 1024 + qc * 512:PK1_DEC + di * 1024 + (qc + 1) * 512],
                        start=(di == 0), stop=(di == 3))
                qt_copy = nc.scalar.copy(qtT[:, hp * S + qc * 512:hp * S + (qc + 1) * 512], p[:])
                p = ps("misc")
                for di in range(4):
                    nc.tensor.matmul(
                        p[:],
                        pk2t[:, PK2_WK + di * 512 + hp * 128:PK2_WK + di * 512 + (hp + 1) * 128],
                        pk2t[:, PK2_MEM + di * 1024 + qc * 512:PK2_MEM + di * 1024 + (qc + 1) * 512],
                        start=(di == 0), stop=(di == 3))
                nc.scalar.copy(ktT[:, hp * S + qc * 512:hp * S + (qc + 1) * 512], p[:])
            return qt_copy

        def proj_v():
            for kt in range(KT):
                p = ps("misc")
                for di in range(4):
                    nc.tensor.matmul(
                        p[:],
                        pk2t[:, PK2_MEM + di * 1024 + kt * 128:PK2_MEM + di * 1024 + (kt + 1) * 128],
                        pk2t[:, PK2_WV + di * 512:PK2_WV + (di + 1) * 512],
                        start=(di == 0), stop=(di == 3))
                dst = vaug[:, kt * H * VW:(kt + 1) * H * VW].rearrange(
                    "p (h c) -> p h c", h=H)[:, :, 0:DH]
                src_ = p[:].rearrange("p (h c) -> p h c", h=H)
                nc.scalar.copy(dst, src_)

        first_qt_copy = proj_qk(0)
        proj_v()

        # pk3 (final-linear weights) reuses pk2's SBUF slot after projections
        pk3t = per.tile([128, PK3_F], F32R, tag="pkshared", name="pk3t")
        pk3_dma = nc.sync.dma_start(pk3t[:], pk3[:].bitcast(F32R))

        # ---- attention per head pair (projections for hp+1 interleaved) --
        for hp in range(HP):
            if hp > 0:
                proj_qk(hp)
            zbs = [work.tile([128, S], BF16, tag="zbs", bufs=4, name=f"zbs_{hp}_{r}")
                   for r in range(2)]
            # U strips for all 8 k-tiles, both heads: kt strip at UOFF[kt],
            # head r at + r*QWIN[kt]
            uh2 = uhp.tile([128, UOFF[KT]], BF16, tag="uh", bufs=2,
                           name=f"uh_{hp}")
            def uhsl(kt, r, qa=None, qb=None):
                qa = QLO[kt] if qa is None else qa
                qb = QLO[kt] + QWIN[kt] if qb is None else qb
                c0 = UOFF[kt] + r * QWIN[kt] + (qa - QLO[kt])
                return uh2[:, c0:c0 + (qb - qa)]
            # scores + exp + E-mult per k-tile over its live q-window
            for kt in range(KT):
                W = QWIN[kt]
                stp = [ps("st"), ps("st")]
                for r in range(2):
                    nc.tensor.matmul(
                        stp[r][:, 0:W],
                        ktT[64 * r:64 * (r + 1), hp * S + kt * 128:hp * S + (kt + 1) * 128],
                        qtT[64 * r:64 * (r + 1), hp * S + QLO[kt]:hp * S + QLO[kt] + W],
                        start=True, stop=True, tile_position=(64 * r, 0))
                for r in range(2):
                    es_i = nc.scalar.activation(
                        uhsl(kt, r), stp[r][:, 0:W],
                        mybir.ActivationFunctionType.Exp, scale=0.125)
                    if hp == 2 and kt == 0 and r == 0:
                        mid_marker = es_i
                    sl = uhsl(kt, r)
                    nc.vector.tensor_mul(
                        sl, sl, ept[:, EPOFF[kt]:EPOFF[kt] + W])
            # AV + Z + normalize per head, per 512-col q-chunk
            for r in range(2):
                h = 2 * hp + r
                for qc in range(QC):
                    av = ps("av", p=VW)
                    kts_qc = sorted({kt for b in range(4 * qc, 4 * qc + 4)
                                     for kt in BKTS[b]})
                    # one accumulation group per bank: the first matmul
                    # zeroes the whole 2KB region (pending-zero), later ones
                    # accumulate per element; one matmul per kt spanning its
                    # contiguous in-chunk block range
                    spans = []
                    for kt in kts_qc:
                        b0 = max(BLO[kt], 4 * qc)
                        b1 = min(BHI[kt], 4 * qc + 3)
                        spans.append((kt, b0, b1))
                    # zero-init the whole [65, 512] region (zero stationary),
                    # then every AV matmul accumulates per element
                    nc.tensor.matmul(av[:], zrow[0:1, :], ept[0:1, 0:512],
                                     start=True, stop=False)
                    for j, (kt, b0, b1) in enumerate(spans):
                        nc.tensor.matmul(
                            av[:, (b0 - 4 * qc) * 128:(b1 - 4 * qc + 1) * 128],
                            vaug[:, kt * H * VW + h * VW:kt * H * VW + (h + 1) * VW],
                            uhsl(kt, r, 128 * b0, 128 * (b1 + 1)),
                            start=False, stop=(j == len(spans) - 1))
                    zr = work.tile([1, 512], F32, tag="zr")
                    nc.vector.reciprocal(zr[:], av[DH:VW, :])
                    zqb = work.tile([1, 512], BF16, tag="zqb")
                    nc.vector.tensor_mul(zqb[:], zr[:], qmt[:, qc * 512:(qc + 1) * 512])
                    # broadcast zq to 128 partitions via tiny PE matmul
                    zbp = ps("misc")
                    nc.tensor.matmul(zbp[:], onesb[0:1, :], zqb[0:1, :],
                                     start=True, stop=True)
                    nc.scalar.copy(zbs[r][:, qc * 512:(qc + 1) * 512], zbp[:])
                    # res.T chunk: catr rows 64r..64r+64 of pair hp
                    avb = work.tile([64, 512], BF16, tag="avb")
                    nc.scalar.copy(avb[:], av[0:DH, :])
                    nc.gpsimd.tensor_mul(
                        catr[64 * r:64 * (r + 1), hp * S + qc * 512:hp * S + (qc + 1) * 512],
                        avb[:], zbs[r][0:64, qc * 512:(qc + 1) * 512])
                # normalized attention out over each k-tile's live window
                for kt in range(KT):
                    W = QWIN[kt]
                    ao = work.tile([128, 384], BF16, tag="ao", bufs=4,
                                   name=f"ao_{h}_{kt}")
                    eng = nc.gpsimd if kt in (0, 4) else nc.vector
                    eng.tensor_mul(ao[:, 0:W], uhsl(kt, r),
                                   zbs[r][:, QLO[kt]:QLO[kt] + W])
                    nc.sync.dma_start(
                        attn_o[h, kt * 128:(kt + 1) * 128, QLO[kt]:QLO[kt] + W],
                        ao[:, 0:W])

        # ---- final linear (+ residual + bias via Wf_aug) + LayerNorm -----
        inv_d = 1.0 / D
        for qt in range(KT):
            p = ps("misc")
            for ct in range(4):
                nc.tensor.matmul(
                    p[:],
                    pk1t[:, PK1_DEC + ct * 1024 + qt * 128:PK1_DEC + ct * 1024 + (qt + 1) * 128],
                    pk3t[:, PK3_WF + ct * 512:PK3_WF + (ct + 1) * 512],
                    start=(ct == 0), stop=False)
            for ct in range(4):
                nc.tensor.matmul(
                    p[:],
                    catr[:, ct * S + qt * 128:ct * S + (qt + 1) * 128],
                    pk3t[:, PK3_WF + (4 + ct) * 512:PK3_WF + (5 + ct) * 512],
                    start=False, stop=False)
            nc.tensor.matmul(p[:], gbt[0:1, 2 * D:2 * D + 128],
                             pk3t[0:1, PK3_BF:PK3_BF + 512],
                             start=False, stop=True)
            # LayerNorm over free axis via fused bn_stats/bn_aggr
            bst = work.tile([128, 6], F32, tag="bst")
            nc.vector.bn_stats(bst[:], p[:])
            mv = work.tile([128, 2], F32, tag="mv")
            nc.vector.bn_aggr(mv[:], bst[:])
            sv = work.tile([128, 1], F32, tag="sv")
            nc.scalar.activation(sv[:], mv[:, 1:2], mybir.ActivationFunctionType.Sqrt,
                                 bias=epst[:])
            rstd = work.tile([128, 1], F32, tag="rstd")
            nc.vector.reciprocal(rstd[:], sv[:])
            xn = work.tile([128, D], F32, tag="xn")
            nc.vector.tensor_scalar(xn[:], p[:], mv[:, 0:1], rstd[:],
                                    mybir.AluOpType.subtract, mybir.AluOpType.mult)
            t1 = work.tile([128, D], F32, tag="t1")
            nc.vector.tensor_mul(t1[:], xn[:], gb[:])
            nc.vector.tensor_add(t1[:], t1[:], bb[:])
            nc.sync.dma_start(out_ln[qt * 128:(qt + 1) * 128, :], t1[:])

    if split:
        _split_multi_waits(nc)
    return nc


_CACHE = {}


def _get_nc():
    if "nc" not in _CACHE:
        _CACHE["nc"] = build()
    return _CACHE["nc"]


def make_in_maps(memory, decoder_input, mask, query_mask, Wk, Wv, Wq, Wf, bf,
                 gamma, beta, gfac):
    memory = np.asarray(memory, dtype=np.float32)
    decoder_input = np.asarray(decoder_input, dtype=np.float32)
    mask = np.asarray(mask)
    query_mask = np.asarray(query_mask, dtype=np.float32)
    gfac = float(np.asarray(gfac))
    # The banded structure (cutoff |k-q| >= 52) is compiled in and derived
    # from gfac=30: exp(-52^2/30) = e^-90 underflows f32. Guard against a
    # different gfac silently producing wrong (truncated) attention.
    assert abs(gfac - 30.0) < 1e-6, (
        f"kernel compiled for gfac=30 (gaussian band cutoff 52), got {gfac}")

    # gaussian factor exp(-(k-q)^2/gfac), [k, q]
    idx = np.arange(S, dtype=np.float64)
    gauss = np.exp(-((idx[:, None] - idx[None, :]) ** 2) / gfac).astype(np.float32)

    WqT = np.ascontiguousarray(np.asarray(Wq, np.float32).T)   # [din, dout]
    WkT = np.ascontiguousarray(np.asarray(Wk, np.float32).T)
    WvT = np.ascontiguousarray(np.asarray(Wv, np.float32).T)
    WfT = np.ascontiguousarray(np.asarray(Wf, np.float32).T)   # [1024, 512]
    Wf_aug = WfT.copy()
    Wf_aug[0:D, :] += np.eye(D, dtype=np.float32)              # residual fold

    def chunks(a, w):
        # [rows, w] -> [128, (rows/128)*w] laid side by side
        r = a.shape[0]
        return np.concatenate([a[i * 128:(i + 1) * 128, :] for i in range(r // 128)],
                              axis=1)

    in_maps = []
    for b in range(B):
        decT = np.ascontiguousarray(decoder_input[b].T)        # [512, 1024]
        memT = np.ascontiguousarray(memory[b].T)
        pk1 = chunks(decT, S)
        bfrow = np.zeros((128, D), np.float32)
        bfrow[0, :] = np.asarray(bf, np.float32)
        pk2 = np.concatenate(
            [chunks(memT, S), chunks(WkT, D), chunks(WvT, D), chunks(WqT, D)], axis=1)
        pk3 = np.concatenate([chunks(Wf_aug[0:1024], D), bfrow], axis=1)
        Eb = np.where(mask[b].T, np.float32(0), gauss)          # [k, q]
        blocks = []
        for kt in range(KT):
            blocks.append(Eb[kt * 128:(kt + 1) * 128, QLO[kt]:QLO[kt] + QWIN[kt]])
        ep = np.concatenate(blocks, axis=1).astype(ml_dtypes.bfloat16)
        qm = query_mask[b][None, :].astype(np.float32)
        gbr = np.concatenate([np.asarray(gamma, np.float32),
                              np.asarray(beta, np.float32),
                              np.ones(128, np.float32)])[None, :]
        in_maps.append({"pk1": np.ascontiguousarray(pk1),
                        "pk2": np.ascontiguousarray(pk2),
                        "pk3": np.ascontiguousarray(pk3),
                        "ep": np.ascontiguousarray(ep),
                        "qm": qm,
                        "gbr": np.ascontiguousarray(gbr)})
    return in_maps


def postprocess(results):
    out = np.empty((B, S, D), np.float32)
    attn = np.empty((B, H, S, S), np.float32)
    for b in range(B):
        out[b] = results[b]["out_ln"]
        # stored [h, k, q] -> reference wants [h, q, k]
        attn[b] = results[b]["attn_o"].astype(np.float32).transpose(0, 2, 1)
    return out, attn


def kernel(**inputs):
    nc = _get_nc()
    in_maps = make_in_maps(**inputs)
    res = run_bass_kernel_spmd(nc, in_maps, core_ids=list(range(N_CORES)))
    return postprocess(res.results)


# revision 58
# speedup vs baseline: 1.0395x; 1.0395x over previous
"""Trainium2 Bass kernel for nn_Attention (dense_transformer).

B=8 batches -> pure data-parallel across 8 NeuronCores (one batch per core,
weights replicated, no collectives).

Per-core dataflow (all matmuls keep seq on the free axis, no on-chip
transposes anywhere):
  - host packs dec.T, mem.T, Wq.T/Wk.T/Wv.T, Wf_aug.T into big f32 DRAM
    tensors; DMAs are chunked so the first projection starts ~2.5us in
  - projections produce Q.T/K.T (fp32r: 4x faster than fp32 on the PE at
    identical observed precision) and V in bf16 with a ones column per head
  - the gaussian bias makes attention banded: exp(-(k-q)^2/30) underflows
    for |k-q| >= 52, so each 128-row k-tile only touches a <=384-wide q
    window (22 of 64 blocks computed; the rest of the attn output stays in
    the runtime's pre-zeroed output buffer)
  - scores S.T[k,q] = K_h @ Q_h.T via fp32r matmuls, two heads packed in
    the PE array with tile_position row groups
  - softmax needs no max subtraction (scores bounded): ACT computes
    exp(0.125*s) straight out of PSUM into the U tile (bf16)
  - DVE multiplies in place by host-precomputed E.T = exp(-gauss)*(1-mask.T)
  - AV matmuls (bf16) contract k on partitions, accumulating per-element
    into a zero-initialized PSUM bank; the ones column lands the softmax
    denominator Z in psum row 64
  - zq = query_mask/Z broadcast to 128 partitions via a tiny PE matmul;
    normalization is a bf16 multiply split across DVE/GPSIMD
  - residual + bias fold into Wf_aug host-side (identity block + bias row);
    final linear is fp32r with cat.T = [dec.T; res.T; ones] stationary
  - LayerNorm via fused bn_stats/bn_aggr over the free axis, f32 out
  - attn written to DRAM as [h, k, q] bf16; host returns a transposed view

A post-pass splits multi-wait instructions into single-wait NoOps (the
walrus build here rejects >1 sync-wait per instruction).
"""
import os
import sys
sys.path.insert(0, "/opt/trn_rl_repo")
os.environ.setdefault("JAX_PLATFORMS", "axon,cpu")

import numpy as np
import ml_dtypes
from contextlib import ExitStack

import concourse.bass as bass
import concourse.tile as tile
from concourse import mybir
from concourse.bass_utils import run_bass_kernel_spmd
from concourse.tile import add_dep_helper

N_CORES = 8
B, S, D, H = 8, 1024, 512, 8
DH = D // H          # 64
HP = H // 2          # 4 head pairs
KT = S // 128        # 8 k tiles
QC = S // 512        # 2 q chunks of 512
LN_EPS = 1e-5

F32 = mybir.dt.float32
F32R = mybir.dt.float32r
BF16 = mybir.dt.bfloat16

# pk1 packed cols: decT (4x1024)
PK1_DEC = 0
PK1_F = 4096
# pk2 packed cols: memT (4x1024) | WkT (4x512) | WvT (4x512) | WqT (4x512)
PK2_MEM = 0
PK2_WK = 4096
PK2_WV = 6144
PK2_WQ = 8192
PK2_F = 10240
# pk3 packed cols: WfT_aug (8x512) | bf row (512)
PK3_WF = 0
PK3_BF = 4096
PK3_F = 4608
VW = DH + 1          # 65: V columns per head incl. ones column
# banding: exp(-(k-q)^2/gfac) underflows f32/bf16 for |k-q| >= 52, so each
# k-tile kt only interacts with q in [128kt-52, 128kt+180). Rounded to
# 128-blocks:
BLO = [max(0, (128 * kt - 52) // 128) for kt in range(KT)]
BHI = [min(KT - 1, (128 * kt + 179) // 128) for kt in range(KT)]
QLO = [128 * b for b in BLO]
QWIN = [128 * (BHI[kt] - BLO[kt] + 1) for kt in range(KT)]     # 256..384
UOFF = [0]
for kt in range(KT):
    UOFF.append(UOFF[-1] + 2 * QWIN[kt])
EPOFF = [0]
for kt in range(KT):
    EPOFF.append(EPOFF[-1] + QWIN[kt])
EP_F = EPOFF[-1]                                               # 2816
# kts covering each q-block, in order
BKTS = [[kt for kt in range(KT) if BLO[kt] <= b <= BHI[kt]] for b in range(KT)]


def _split_multi_waits(nc):
    """walrus in this container rejects >1 sync-wait per instruction.
    Hoist extra waits onto single-wait NoOps inserted just before."""
    import bass_rust
    cnt = 0
    for f in nc.m.functions:
        for blk in f.blocks:
            new_insts = []
            for inst in blk.instructions:
                si = getattr(inst, "sync_info", None)
                if si is not None and si.on_wait and len(si.on_wait) > 1:
                    waits = list(si.on_wait)
                    for w in waits[:-1]:
                        cnt += 1
                        nop = mybir.InstNoOp(name=f"W-split-{cnt}", ins=[], outs=[])
                        nop.engine = inst.engine
                        nop.sync_info = bass_rust.SyncInfo(on_wait=[w], on_update=[])
                        new_insts.append(nop)
                    inst.sync_info = bass_rust.SyncInfo(
                        on_wait=[waits[-1]], on_update=list(si.on_update))
                new_insts.append(inst)
            blk.instructions = new_insts
    return cnt


def build(split=True):
    nc = bass.Bass("TRN2", target_bir_lowering=False, debug=False,
                   num_devices=N_CORES)
    pk1 = nc.dram_tensor("pk1", [128, PK1_F], F32, kind="ExternalInput").ap()
    pk2 = nc.dram_tensor("pk2", [128, PK2_F], F32, kind="ExternalInput").ap()
    pk3 = nc.dram_tensor("pk3", [128, PK3_F], F32, kind="ExternalInput").ap()
    epd = nc.dram_tensor("ep", [128, EP_F], BF16, kind="ExternalInput").ap()
    qmd = nc.dram_tensor("qm", [1, S], F32, kind="ExternalInput").ap()
    gbd = nc.dram_tensor("gbr", [1, 2 * D + 128], F32, kind="ExternalInput").ap()
    out_ln = nc.dram_tensor("out_ln", [S, D], F32, kind="ExternalOutput").ap()
    attn_o = nc.dram_tensor("attn_o", [H, S, S], BF16, kind="ExternalOutput").ap()

    with tile.TileContext(nc) as tc, ExitStack() as ctx:
        per = ctx.enter_context(tc.tile_pool(name="per", bufs=1))
        work = ctx.enter_context(tc.tile_pool(name="work", bufs=2))
        uhp = ctx.enter_context(tc.tile_pool(name="uhp", bufs=1))
        psum = ctx.enter_context(tc.tile_pool(name="psum", bufs=1, space="PSUM"))

        # ---- persistent tiles / input DMAs -------------------------------
        qmt = per.tile([1, S], F32)
        nc.sync.dma_start(qmt[:], qmd[:])
        gbt = per.tile([1, 2 * D + 128], F32R)
        nc.sync.dma_start(gbt[:], gbd[:].bitcast(F32R))
        pk1t = per.tile([128, PK1_F], F32R)
        pk2t = per.tile([128, PK2_F], F32R, tag="pkshared")
        for di in range(4):
            nc.sync.dma_start(pk2t[:, PK2_WQ + di * 512:PK2_WQ + (di + 1) * 512],
                              pk2[:, PK2_WQ + di * 512:PK2_WQ + (di + 1) * 512].bitcast(F32R))
            lo = PK1_DEC + di * 1024
            nc.sync.dma_start(pk1t[:, lo:lo + 1024],
                              pk1[:, lo:lo + 1024].bitcast(F32R))
        for di in range(4):
            nc.sync.dma_start(pk2t[:, PK2_WK + di * 512:PK2_WK + (di + 1) * 512],
                              pk2[:, PK2_WK + di * 512:PK2_WK + (di + 1) * 512].bitcast(F32R))
            lo = PK2_MEM + di * 1024
            nc.sync.dma_start(pk2t[:, lo:lo + 1024],
                              pk2[:, lo:lo + 1024].bitcast(F32R))
        for di in range(4):
            nc.sync.dma_start(pk2t[:, PK2_WV + di * 512:PK2_WV + (di + 1) * 512],
                              pk2[:, PK2_WV + di * 512:PK2_WV + (di + 1) * 512].bitcast(F32R))
        ept = per.tile([128, EP_F], BF16)
        ep_dma = nc.sync.dma_start(ept[:], epd[:])

        qtT = per.tile([128, HP * S], F32R)     # Q.T packed, pair hp at cols hp*1024
        ktT = per.tile([128, HP * S], F32R)     # K.T packed
        vaug = per.tile([128, KT * H * VW], BF16)  # V + ones col, ktile kt at cols kt*520
        catr = per.tile([128, HP * S], F32R)    # res.T packed (cat rows 512..1023)
        onesb = per.tile([1, 128], BF16)        # bf16 ones row
        zrow = per.tile([1, VW], BF16)          # bf16 zeros row (psum init)
        gb = per.tile([128, D], BF16)           # gamma broadcast (bf16 apply)
        bb = per.tile([128, D], BF16)           # beta broadcast
        epst = per.tile([128, 1], F32)          # LN eps per-partition const

        nc.gpsimd.memset(vaug[:], 1.0)
        nc.vector.memset(epst[:], LN_EPS)
        nc.gpsimd.memset(onesb[:], 1.0)
        nc.gpsimd.memset(zrow[:], 0.0)

        _psn = [0]
        PS_BUFS = {"st": 4, "av": 2, "misc": 2}
        def ps(tag, p=128, n=512):
            _psn[0] += 1
            return psum.tile([p, n], F32, tag=tag, bufs=PS_BUFS[tag],
                             name=f"ps_{tag}_{_psn[0]}")

        # attn regions outside the gaussian band stay zero: ExternalOutput
        # buffers are pre-zeroed by the runtime, so nothing to write there.

        # ---- gamma/beta broadcast to 128 partitions ----------------------
        for i, dst in enumerate((gb, bb)):
            p = ps("misc")
            nc.tensor.matmul(p[:], gbt[0:1, 2 * D:2 * D + 128], gbt[0:1, i * D:(i + 1) * D],
                             start=True, stop=True)
            nc.scalar.copy(dst[:], p[:])

        # ---- projections: Q.T, K.T (f32r), V -> vaug (bf16) --------------
        def proj_qk(hp):
            for qc in range(QC):
                p = ps("misc")
                for di in range(4):
                    nc.tensor.matmul(
                        p[:],
                        pk2t[:, PK2_WQ + di * 512 + hp * 128:PK2_WQ + di * 512 + (hp + 1) * 128],
                        pk1t[:, PK1_DEC + di * 1024 + qc * 512:PK1_DEC + di * 1024 + (qc + 1) * 512],
                        start=(di == 0), stop=(di == 3))
                qt_copy = nc.scalar.copy(qtT[:, hp * S + qc * 512:hp * S + (qc + 1) * 512], p[:])
                p = ps("misc")
                for di in range(4):
                    nc.tensor.matmul(
                        p[:],
                        pk2t[:, PK2_WK + di * 512 + hp * 128:PK2_WK + di * 512 + (hp + 1) * 128],
                        pk2t[:, PK2_MEM + di * 1024 + qc * 512:PK2_MEM + di * 1024 + (qc + 1) * 512],
                        start=(di == 0), stop=(di == 3))
                nc.scalar.copy(ktT[:, hp * S + qc * 512:hp * S + (qc + 1) * 512], p[:])
            return qt_copy

        def proj_v():
            for kt in range(KT):
                p = ps("misc")
                for di in range(4):
                    nc.tensor.matmul(
                        p[:],
                        pk2t[:, PK2_MEM + di * 1024 + kt * 128:PK2_MEM + di * 1024 + (kt + 1) * 128],
                        pk2t[:, PK2_WV + di * 512:PK2_WV + (di + 1) * 512],
                        start=(di == 0), stop=(di == 3))
                dst = vaug[:, kt * H * VW:(kt + 1) * H * VW].rearrange(
                    "p (h c) -> p h c", h=H)[:, :, 0:DH]
                src_ = p[:].rearrange("p (h c) -> p h c", h=H)
                nc.scalar.copy(dst, src_)

        first_qt_copy = proj_qk(0)
        proj_v()

        # pk3 (final-linear weights) reuses pk2's SBUF slot after projections
        pk3t = per.tile([128, PK3_F], F32R, tag="pkshared", name="pk3t")
        pk3_dma = nc.sync.dma_start(pk3t[:], pk3[:].bitcast(F32R))

        # ---- attention per head pair (projections for hp+1 interleaved) --
        for hp in range(HP):
            if hp > 0:
                proj_qk(hp)
            zbs = [work.tile([128, S], BF16, tag="zbs", bufs=4, name=f"zbs_{hp}_{r}")
                   for r in range(2)]
            # U strips for all 8 k-tiles, both heads: kt strip at UOFF[kt],
            # head r at + r*QWIN[kt]
            uh2 = uhp.tile([128, UOFF[KT]], BF16, tag="uh", bufs=2,
                           name=f"uh_{hp}")
            def uhsl(kt, r, qa=None, qb=None):
                qa = QLO[kt] if qa is None else qa
                qb = QLO[kt] + QWIN[kt] if qb is None else qb
                c0 = UOFF[kt] + r * QWIN[kt] + (qa - QLO[kt])
                return uh2[:, c0:c0 + (qb - qa)]
            # scores + exp + E-mult per k-tile over its live q-window
            for kt in range(KT):
                W = QWIN[kt]
                stp = [ps("st"), ps("st")]
                for r in range(2):
                    nc.tensor.matmul(
                        stp[r][:, 0:W],
                        ktT[64 * r:64 * (r + 1), hp * S + kt * 128:hp * S + (kt + 1) * 128],
                        qtT[64 * r:64 * (r + 1), hp * S + QLO[kt]:hp * S + QLO[kt] + W],
                        start=True, stop=True, tile_position=(64 * r, 0))
                for r in range(2):
                    es_i = nc.scalar.activation(
                        uhsl(kt, r), stp[r][:, 0:W],
                        mybir.ActivationFunctionType.Exp, scale=0.125)
                    if hp == 2 and kt == 0 and r == 0:
                        mid_marker = es_i
                    sl = uhsl(kt, r)
                    nc.vector.tensor_mul(
                        sl, sl, ept[:, EPOFF[kt]:EPOFF[kt] + W])
            # AV + Z + normalize per head, per 512-col q-chunk
            for r in range(2):
                h = 2 * hp + r
                for qc in range(QC):
                    av = ps("av", p=VW)
                    kts_qc = sorted({kt for b in range(4 * qc, 4 * qc + 4)
                                     for kt in BKTS[b]})
                    # one accumulation group per bank: the first matmul
                    # zeroes the whole 2KB region (pending-zero), later ones
                    # accumulate per element; one matmul per kt spanning its
                    # contiguous in-chunk block range
                    spans = []
                    for kt in kts_qc:
                        b0 = max(BLO[kt], 4 * qc)
                        b1 = min(BHI[kt], 4 * qc + 3)
                        spans.append((kt, b0, b1))
                    # zero-init the whole [65, 512] region (zero stationary),
                    # then every AV matmul accumulates per element
                    nc.tensor.matmul(av[:], zrow[0:1, :], ept[0:1, 0:512],
                                     start=True, stop=False)
                    for j, (kt, b0, b1) in enumerate(spans):
                        nc.tensor.matmul(
                            av[:, (b0 - 4 * qc) * 128:(b1 - 4 * qc + 1) * 128],
                            vaug[:, kt * H * VW + h * VW:kt * H * VW + (h + 1) * VW],
                            uhsl(kt, r, 128 * b0, 128 * (b1 + 1)),
                            start=False, stop=(j == len(spans) - 1))
                    zr = work.tile([1, 512], F32, tag="zr")
                    nc.vector.reciprocal(zr[:], av[DH:VW, :])
                    zqb = work.tile([1, 512], BF16, tag="zqb")
                    nc.vector.tensor_mul(zqb[:], zr[:], qmt[:, qc * 512:(qc + 1) * 512])
                    # broadcast zq to 128 partitions via tiny PE matmul
                    zbp = ps("misc")
                    nc.tensor.matmul(zbp[:], onesb[0:1, :], zqb[0:1, :],
                                     start=True, stop=True)
                    nc.scalar.copy(zbs[r][:, qc * 512:(qc + 1) * 512], zbp[:])
                    # res.T chunk: catr rows 64r..64r+64 of pair hp
                    avb = work.tile([64, 512], BF16, tag="avb")
                    nc.scalar.copy(avb[:], av[0:DH, :])
                    nc.gpsimd.tensor_mul(
                        catr[64 * r:64 * (r + 1), hp * S + qc * 512:hp * S + (qc + 1) * 512],
                        avb[:], zbs[r][0:64, qc * 512:(qc + 1) * 512])
                # normalized attention out over each k-tile's live window
                for kt in range(KT):
                    W = QWIN[kt]
                    ao = work.tile([128, 384], BF16, tag="ao", bufs=4,
                                   name=f"ao_{h}_{kt}")
                    eng = nc.gpsimd if kt in (0, 4) else nc.vector
                    eng.tensor_mul(ao[:, 0:W], uhsl(kt, r),
                                   zbs[r][:, QLO[kt]:QLO[kt] + W])
                    nc.sync.dma_start(
                        attn_o[h, kt * 128:(kt + 1) * 128, QLO[kt]:QLO[kt] + W],
                        ao[:, 0:W])

        # ---- final linear (+ residual + bias via Wf_aug) + LayerNorm -----
        inv_d = 1.0 / D
        for qt in range(KT):
            p = ps("st")
            for ct in range(4):
                nc.tensor.matmul(
                    p[:],
                    pk1t[:, PK1_DEC + ct * 1024 + qt * 128:PK1_DEC + ct * 1024 + (qt + 1) * 128],
                    pk3t[:, PK3_WF + ct * 512:PK3_WF + (ct + 1) * 512],
                    start=(ct == 0), stop=False)
            for ct in range(4):
                nc.tensor.matmul(
                    p[:],
                    catr[:, ct * S + qt * 128:ct * S + (qt + 1) * 128],
                    pk3t[:, PK3_WF + (4 + ct) * 512:PK3_WF + (5 + ct) * 512],
                    start=False, stop=False)
            nc.tensor.matmul(p[:], gbt[0:1, 2 * D:2 * D + 128],
                             pk3t[0:1, PK3_BF:PK3_BF + 512],
                             start=False, stop=True)
            # LayerNorm over free axis via fused bn_stats/bn_aggr
            bst = work.tile([128, 6], F32, tag="bst")
            nc.vector.bn_stats(bst[:], p[:])
            mv = work.tile([128, 2], F32, tag="mv")
            nc.vector.bn_aggr(mv[:], bst[:])
            sv = work.tile([128, 1], F32, tag="sv")
            nc.scalar.activation(sv[:], mv[:, 1:2], mybir.ActivationFunctionType.Sqrt,
                                 bias=epst[:])
            rstd = work.tile([128, 1], F32, tag="rstd")
            nc.vector.reciprocal(rstd[:], sv[:])
            mb = work.tile([128, 1], F32, tag="mb")
            nc.vector.tensor_scalar(mb[:], mv[:, 0:1], rstd[:], -1.0,
                                    mybir.AluOpType.mult, mybir.AluOpType.mult)
            xn = work.tile([128, D], BF16, tag="xn")
            nc.scalar.activation(xn[:], p[:], mybir.ActivationFunctionType.Identity,
                                 bias=mb[:], scale=rstd[:])
            xg = work.tile([128, D], BF16, tag="xg")
            nc.vector.tensor_mul(xg[:], xn[:], gb[:])
            t1 = work.tile([128, D], F32, tag="t1")
            nc.vector.tensor_add(t1[:], xg[:], bb[:])
            nc.sync.dma_start(out_ln[qt * 128:(qt + 1) * 128, :], t1[:])

    if split:
        _split_multi_waits(nc)
    return nc


_CACHE = {}


def _get_nc():
    if "nc" not in _CACHE:
        _CACHE["nc"] = build()
    return _CACHE["nc"]


def make_in_maps(memory, decoder_input, mask, query_mask, Wk, Wv, Wq, Wf, bf,
                 gamma, beta, gfac):
    memory = np.asarray(memory, dtype=np.float32)
    decoder_input = np.asarray(decoder_input, dtype=np.float32)
    mask = np.asarray(mask)
    query_mask = np.asarray(query_mask, dtype=np.float32)
    gfac = float(np.asarray(gfac))
    # The banded structure (cutoff |k-q| >= 52) is compiled in and derived
    # from gfac=30: exp(-52^2/30) = e^-90 underflows f32. Guard against a
    # different gfac silently producing wrong (truncated) attention.
    assert abs(gfac - 30.0) < 1e-6, (
        f"kernel compiled for gfac=30 (gaussian band cutoff 52), got {gfac}")

    # gaussian factor exp(-(k-q)^2/gfac), [k, q]
    idx = np.arange(S, dtype=np.float64)
    gauss = np.exp(-((idx[:, None] - idx[None, :]) ** 2) / gfac).astype(np.float32)

    WqT = np.ascontiguousarray(np.asarray(Wq, np.float32).T)   # [din, dout]
    WkT = np.ascontiguousarray(np.asarray(Wk, np.float32).T)
    WvT = np.ascontiguousarray(np.asarray(Wv, np.float32).T)
    WfT = np.ascontiguousarray(np.asarray(Wf, np.float32).T)   # [1024, 512]
    Wf_aug = WfT.copy()
    Wf_aug[0:D, :] += np.eye(D, dtype=np.float32)              # residual fold

    def chunks(a, w):
        # [rows, w] -> [128, (rows/128)*w] laid side by side
        r = a.shape[0]
        return np.concatenate([a[i * 128:(i + 1) * 128, :] for i in range(r // 128)],
                              axis=1)

    in_maps = []
    for b in range(B):
        decT = np.ascontiguousarray(decoder_input[b].T)        # [512, 1024]
        memT = np.ascontiguousarray(memory[b].T)
        pk1 = chunks(decT, S)
        bfrow = np.zeros((128, D), np.float32)
        bfrow[0, :] = np.asarray(bf, np.float32)
        pk2 = np.concatenate(
            [chunks(memT, S), chunks(WkT, D), chunks(WvT, D), chunks(WqT, D)], axis=1)
        pk3 = np.concatenate([chunks(Wf_aug[0:1024], D), bfrow], axis=1)
        Eb = np.where(mask[b].T, np.float32(0), gauss)          # [k, q]
        blocks = []
        for kt in range(KT):
            blocks.append(Eb[kt * 128:(kt + 1) * 128, QLO[kt]:QLO[kt] + QWIN[kt]])
        ep = np.concatenate(blocks, axis=1).astype(ml_dtypes.bfloat16)
        qm = query_mask[b][None, :].astype(np.float32)
        gbr = np.concatenate([np.asarray(gamma, np.float32),
                              np.asarray(beta, np.float32),
                              np.ones(128, np.float32)])[None, :]
        in_maps.append({"pk1": np.ascontiguousarray(pk1),
                        "pk2": np.ascontiguousarray(pk2),
                        "pk3": np.ascontiguousarray(pk3),
                        "ep": np.ascontiguousarray(ep),
                        "qm": qm,
                        "gbr": np.ascontiguousarray(gbr)})
    return in_maps


def postprocess(results):
    out = np.empty((B, S, D), np.float32)
    attn = np.empty((B, H, S, S), np.float32)
    for b in range(B):
        out[b] = results[b]["out_ln"]
        # stored [h, k, q] -> reference wants [h, q, k]
        attn[b] = results[b]["attn_o"].astype(np.float32).transpose(0, 2, 1)
    return out, attn


def kernel(**inputs):
    nc = _get_nc()
    in_maps = make_in_maps(**inputs)
    res = run_bass_kernel_spmd(nc, in_maps, core_ids=list(range(N_CORES)))
    return postprocess(res.results)


# revision 65
# speedup vs baseline: 1.1194x; 1.0769x over previous
"""Trainium2 Bass kernel for nn_Attention (dense_transformer).

B=8 batches -> pure data-parallel across 8 NeuronCores (one batch per core,
weights replicated, no collectives).

Per-core dataflow (all matmuls keep seq on the free axis, no on-chip
transposes anywhere):
  - host packs dec.T, mem.T, Wq.T/Wk.T/Wv.T, Wf_aug.T into big f32 DRAM
    tensors; DMAs are chunked so the first projection starts ~2.5us in
  - projections produce Q.T/K.T (fp32r: 4x faster than fp32 on the PE at
    identical observed precision) and V in bf16 with a ones column per head
  - the gaussian bias makes attention banded: exp(-(k-q)^2/30) underflows
    for |k-q| >= 52, so each 128-row k-tile only touches a <=384-wide q
    window (22 of 64 blocks computed; the rest of the attn output stays in
    the runtime's pre-zeroed output buffer)
  - scores S.T[k,q] = K_h @ Q_h.T via fp32r matmuls, two heads packed in
    the PE array with tile_position row groups
  - softmax needs no max subtraction (scores bounded): ACT computes
    exp(0.125*s) straight out of PSUM into the U tile (bf16)
  - DVE multiplies in place by host-precomputed E.T = exp(-gauss)*(1-mask.T)
  - AV matmuls (bf16) contract k on partitions, accumulating per-element
    into a zero-initialized PSUM bank; the ones column lands the softmax
    denominator Z in psum row 64
  - zq = query_mask/Z broadcast to 128 partitions via a tiny PE matmul;
    normalization is a bf16 multiply split across DVE/GPSIMD
  - residual + bias fold into Wf_aug host-side (identity block + bias row);
    final linear is fp32r with cat.T = [dec.T; res.T; ones] stationary
  - LayerNorm via fused bn_stats/bn_aggr; the (x-mu)*rstd normalize runs
    on ACT (activation Identity with per-partition scale/bias), gamma/beta
    apply in bf16
  - attn written to DRAM as [h, k, q] bf16; host returns a transposed view

A post-pass splits multi-wait instructions into single-wait NoOps (the
walrus build here rejects >1 sync-wait per instruction).
"""
import os
import sys
sys.path.insert(0, "/opt/trn_rl_repo")
os.environ.setdefault("JAX_PLATFORMS", "axon,cpu")

import numpy as np
import ml_dtypes
from contextlib import ExitStack

import concourse.bass as bass
import concourse.tile as tile
from concourse import mybir
from concourse.bass_utils import run_bass_kernel_spmd
from concourse.tile import add_dep_helper

N_CORES = 8
B, S, D, H = 8, 1024, 512, 8
DH = D // H          # 64
HP = H // 2          # 4 head pairs
KT = S // 128        # 8 k tiles
QC = S // 512        # 2 q chunks of 512
LN_EPS = 1e-5

F32 = mybir.dt.float32
F32R = mybir.dt.float32r
BF16 = mybir.dt.bfloat16

# pk1 packed cols: decT (4x1024)
PK1_DEC = 0
PK1_F = 4096
# pk2 packed cols: memT (4x1024) | WkT (4x512) | WvT (4x512) | WqT (4x512)
PK2_MEM = 0
PK2_WK = 4096
PK2_WV = 6144
PK2_WQ = 8192
PK2_F = 10240
# pk3 packed cols: WfT_aug (8x512) | bf row (512)
PK3_WF = 0
PK3_BF = 4096
PK3_F = 4608
VW = DH + 1          # 65: V columns per head incl. ones column
# banding: exp(-(k-q)^2/gfac) underflows f32/bf16 for |k-q| >= 52, so each
# k-tile kt only interacts with q in [128kt-52, 128kt+180). Rounded to
# 128-blocks:
BLO = [max(0, (128 * kt - 52) // 128) for kt in range(KT)]
BHI = [min(KT - 1, (128 * kt + 179) // 128) for kt in range(KT)]
QLO = [128 * b for b in BLO]
QWIN = [128 * (BHI[kt] - BLO[kt] + 1) for kt in range(KT)]     # 256..384
UOFF = [0]
for kt in range(KT):
    UOFF.append(UOFF[-1] + 2 * QWIN[kt])
EPOFF = [0]
for kt in range(KT):
    EPOFF.append(EPOFF[-1] + QWIN[kt])
EP_F = EPOFF[-1]                                               # 2816
# kts covering each q-block, in order
BKTS = [[kt for kt in range(KT) if BLO[kt] <= b <= BHI[kt]] for b in range(KT)]


def _split_multi_waits(nc):
    """walrus in this container rejects >1 sync-wait per instruction.
    Hoist extra waits onto single-wait NoOps inserted just before."""
    import bass_rust
    cnt = 0
    for f in nc.m.functions:
        for blk in f.blocks:
            new_insts = []
            for inst in blk.instructions:
                si = getattr(inst, "sync_info", None)
                if si is not None and si.on_wait and len(si.on_wait) > 1:
                    waits = list(si.on_wait)
                    for w in waits[:-1]:
                        cnt += 1
                        nop = mybir.InstNoOp(name=f"W-split-{cnt}", ins=[], outs=[])
                        nop.engine = inst.engine
                        nop.sync_info = bass_rust.SyncInfo(on_wait=[w], on_update=[])
                        new_insts.append(nop)
                    inst.sync_info = bass_rust.SyncInfo(
                        on_wait=[waits[-1]], on_update=list(si.on_update))
                new_insts.append(inst)
            blk.instructions = new_insts
    return cnt


def build(split=True):
    nc = bass.Bass("TRN2", target_bir_lowering=False, debug=False,
                   num_devices=N_CORES)
    pk1 = nc.dram_tensor("pk1", [128, PK1_F], F32, kind="ExternalInput").ap()
    pk2 = nc.dram_tensor("pk2", [128, PK2_F], F32, kind="ExternalInput").ap()
    pk3 = nc.dram_tensor("pk3", [128, PK3_F], F32, kind="ExternalInput").ap()
    epd = nc.dram_tensor("ep", [128, EP_F], BF16, kind="ExternalInput").ap()
    qmd = nc.dram_tensor("qm", [1, S], F32, kind="ExternalInput").ap()
    gbd = nc.dram_tensor("gbr", [1, 2 * D + 128], F32, kind="ExternalInput").ap()
    out_ln = nc.dram_tensor("out_ln", [S, D], F32, kind="ExternalOutput").ap()
    attn_o = nc.dram_tensor("attn_o", [H, S, S], BF16, kind="ExternalOutput").ap()

    with tile.TileContext(nc) as tc, ExitStack() as ctx:
        per = ctx.enter_context(tc.tile_pool(name="per", bufs=1))
        work = ctx.enter_context(tc.tile_pool(name="work", bufs=2))
        uhp = ctx.enter_context(tc.tile_pool(name="uhp", bufs=1))
        psum = ctx.enter_context(tc.tile_pool(name="psum", bufs=1, space="PSUM"))

        # ---- persistent tiles / input DMAs -------------------------------
        qmt = per.tile([1, S], F32)
        nc.sync.dma_start(qmt[:], qmd[:])
        gbt = per.tile([1, 2 * D + 128], F32R)
        nc.sync.dma_start(gbt[:], gbd[:].bitcast(F32R))
        pk1t = per.tile([128, PK1_F], F32R)
        pk2t = per.tile([128, PK2_F], F32R, tag="pkshared")
        # q-half-granular input streaming: qc0 projections start after ~2.5MB
        for di in range(4):
            nc.sync.dma_start(pk2t[:, PK2_WQ + di * 512:PK2_WQ + (di + 1) * 512],
                              pk2[:, PK2_WQ + di * 512:PK2_WQ + (di + 1) * 512].bitcast(F32R))
            lo = PK1_DEC + di * 1024
            nc.sync.dma_start(pk1t[:, lo:lo + 512],
                              pk1[:, lo:lo + 512].bitcast(F32R))
        for di in range(4):
            nc.sync.dma_start(pk2t[:, PK2_WK + di * 512:PK2_WK + (di + 1) * 512],
                              pk2[:, PK2_WK + di * 512:PK2_WK + (di + 1) * 512].bitcast(F32R))
            lo = PK2_MEM + di * 1024
            nc.sync.dma_start(pk2t[:, lo:lo + 512],
                              pk2[:, lo:lo + 512].bitcast(F32R))
        for di in range(4):
            lo = PK1_DEC + di * 1024
            nc.sync.dma_start(pk1t[:, lo + 512:lo + 1024],
                              pk1[:, lo + 512:lo + 1024].bitcast(F32R))
            lo = PK2_MEM + di * 1024
            nc.sync.dma_start(pk2t[:, lo + 512:lo + 1024],
                              pk2[:, lo + 512:lo + 1024].bitcast(F32R))
        for di in range(4):
            nc.sync.dma_start(pk2t[:, PK2_WV + di * 512:PK2_WV + (di + 1) * 512],
                              pk2[:, PK2_WV + di * 512:PK2_WV + (di + 1) * 512].bitcast(F32R))
        ept = per.tile([128, EP_F], BF16)
        ep_dma = nc.sync.dma_start(ept[:], epd[:])

        qtT = per.tile([128, HP * S], F32R)     # Q.T packed, pair hp at cols hp*1024
        ktT = per.tile([128, HP * S], F32R)     # K.T packed
        vaug = per.tile([128, KT * H * VW], BF16)  # V + ones col, ktile kt at cols kt*520
        catr = per.tile([128, HP * S], F32R)    # res.T packed (cat rows 512..1023)
        onesb = per.tile([1, 128], BF16)        # bf16 ones row
        zrow = per.tile([1, VW], BF16)          # bf16 zeros row (psum init)
        gb = per.tile([128, D], BF16)           # gamma broadcast (bf16 apply)
        bb = per.tile([128, D], BF16)           # beta broadcast
        epst = per.tile([128, 1], F32)          # LN eps per-partition const

        nc.gpsimd.memset(vaug[:], 1.0)
        nc.vector.memset(epst[:], LN_EPS)
        nc.gpsimd.memset(onesb[:], 1.0)
        nc.gpsimd.memset(zrow[:], 0.0)

        _psn = [0]
        PS_BUFS = {"st": 4, "av": 2, "misc": 2}
        def ps(tag, p=128, n=512):
            _psn[0] += 1
            return psum.tile([p, n], F32, tag=tag, bufs=PS_BUFS[tag],
                             name=f"ps_{tag}_{_psn[0]}")

        # attn regions outside the gaussian band stay zero: ExternalOutput
        # buffers are pre-zeroed by the runtime, so nothing to write there.

        # ---- gamma/beta broadcast to 128 partitions ----------------------
        for i, dst in enumerate((gb, bb)):
            p = ps("misc")
            nc.tensor.matmul(p[:], gbt[0:1, 2 * D:2 * D + 128], gbt[0:1, i * D:(i + 1) * D],
                             start=True, stop=True)
            nc.scalar.copy(dst[:], p[:])

        # ---- projections: Q.T, K.T (f32r), V -> vaug (bf16) --------------
        def proj_qk(hp, qcs=(0, 1)):
            for qc in qcs:
                p = ps("st")
                for di in range(4):
                    nc.tensor.matmul(
                        p[:],
                        pk2t[:, PK2_WQ + di * 512 + hp * 128:PK2_WQ + di * 512 + (hp + 1) * 128],
                        pk1t[:, PK1_DEC + di * 1024 + qc * 512:PK1_DEC + di * 1024 + (qc + 1) * 512],
                        start=(di == 0), stop=(di == 3))
                qt_copy = nc.scalar.copy(qtT[:, hp * S + qc * 512:hp * S + (qc + 1) * 512], p[:])
                p = ps("st")
                for di in range(4):
                    nc.tensor.matmul(
                        p[:],
                        pk2t[:, PK2_WK + di * 512 + hp * 128:PK2_WK + di * 512 + (hp + 1) * 128],
                        pk2t[:, PK2_MEM + di * 1024 + qc * 512:PK2_MEM + di * 1024 + (qc + 1) * 512],
                        start=(di == 0), stop=(di == 3))
                nc.scalar.copy(ktT[:, hp * S + qc * 512:hp * S + (qc + 1) * 512], p[:])
            return qt_copy

        def proj_v():
            for kt in range(KT):
                p = ps("st")
                for di in range(4):
                    nc.tensor.matmul(
                        p[:],
                        pk2t[:, PK2_MEM + di * 1024 + kt * 128:PK2_MEM + di * 1024 + (kt + 1) * 128],
                        pk2t[:, PK2_WV + di * 512:PK2_WV + (di + 1) * 512],
                        start=(di == 0), stop=(di == 3))
                dst = vaug[:, kt * H * VW:(kt + 1) * H * VW].rearrange(
                    "p (h c) -> p h c", h=H)[:, :, 0:DH]
                src_ = p[:].rearrange("p (h c) -> p h c", h=H)
                nc.scalar.copy(dst, src_)

        first_qt_copy = proj_qk(0, qcs=(0,))

        # pk3 (final-linear weights) reuses pk2's SBUF slot after projections
        pk3t = per.tile([128, PK3_F], F32R, tag="pkshared", name="pk3t")
        pk3_dma = nc.sync.dma_start(pk3t[:], pk3[:].bitcast(F32R))

        # ---- attention per head pair (projections for hp+1 interleaved) --
        for hp in range(HP):
            if hp > 0:
                proj_qk(hp)
            zbs = [work.tile([128, S], BF16, tag="zbs", bufs=4, name=f"zbs_{hp}_{r}")
                   for r in range(2)]
            # U strips for all 8 k-tiles, both heads: kt strip at UOFF[kt],
            # head r at + r*QWIN[kt]
            uh2 = uhp.tile([128, UOFF[KT]], BF16, tag="uh", bufs=2,
                           name=f"uh_{hp}")
            def uhsl(kt, r, qa=None, qb=None):
                qa = QLO[kt] if qa is None else qa
                qb = QLO[kt] + QWIN[kt] if qb is None else qb
                c0 = UOFF[kt] + r * QWIN[kt] + (qa - QLO[kt])
                return uh2[:, c0:c0 + (qb - qa)]
            # scores + exp + E-mult per k-tile over its live q-window
            for kt in range(KT):
                W = QWIN[kt]
                stp = [ps("st"), ps("st")]
                for r in range(2):
                    nc.tensor.matmul(
                        stp[r][:, 0:W],
                        ktT[64 * r:64 * (r + 1), hp * S + kt * 128:hp * S + (kt + 1) * 128],
                        qtT[64 * r:64 * (r + 1), hp * S + QLO[kt]:hp * S + QLO[kt] + W],
                        start=True, stop=True, tile_position=(64 * r, 0))
                for r in range(2):
                    es_i = nc.scalar.activation(
                        uhsl(kt, r), stp[r][:, 0:W],
                        mybir.ActivationFunctionType.Exp, scale=0.125)
                    if hp == 2 and kt == 0 and r == 0:
                        mid_marker = es_i
                    sl = uhsl(kt, r)
                    nc.vector.tensor_mul(
                        sl, sl, ept[:, EPOFF[kt]:EPOFF[kt] + W])
                if hp == 0 and kt == 2:
                    # stream-phase: qc1 projections + V only now, so the
                    # first three score tiles start after just the qc0 data
                    proj_qk(0, qcs=(1,))
                    proj_v()
            # AV + Z + normalize per head, per 512-col q-chunk
            for r in range(2):
                h = 2 * hp + r
                for qc in range(QC):
                    av = ps("av", p=VW)
                    kts_qc = sorted({kt for b in range(4 * qc, 4 * qc + 4)
                                     for kt in BKTS[b]})
                    # one accumulation group per bank: the first matmul
                    # zeroes the whole 2KB region (pending-zero), later ones
                    # accumulate per element; one matmul per kt spanning its
                    # contiguous in-chunk block range
                    spans = []
                    for kt in kts_qc:
                        b0 = max(BLO[kt], 4 * qc)
                        b1 = min(BHI[kt], 4 * qc + 3)
                        spans.append((kt, b0, b1))
                    # zero-init the whole [65, 512] region (zero stationary),
                    # then every AV matmul accumulates per element
                    nc.tensor.matmul(av[:], zrow[0:1, :], ept[0:1, 0:512],
                                     start=True, stop=False)
                    for j, (kt, b0, b1) in enumerate(spans):
                        nc.tensor.matmul(
                            av[:, (b0 - 4 * qc) * 128:(b1 - 4 * qc + 1) * 128],
                            vaug[:, kt * H * VW + h * VW:kt * H * VW + (h + 1) * VW],
                            uhsl(kt, r, 128 * b0, 128 * (b1 + 1)),
                            start=False, stop=(j == len(spans) - 1))
                    zr = work.tile([1, 512], F32, tag="zr")
                    nc.vector.reciprocal(zr[:], av[DH:VW, :])
                    zqb = work.tile([1, 512], BF16, tag="zqb")
                    nc.vector.tensor_mul(zqb[:], zr[:], qmt[:, qc * 512:(qc + 1) * 512])
                    # broadcast zq to 128 partitions via tiny PE matmul
                    zbp = ps("misc")
                    nc.tensor.matmul(zbp[:], onesb[0:1, :], zqb[0:1, :],
                                     start=True, stop=True)
                    nc.scalar.copy(zbs[r][:, qc * 512:(qc + 1) * 512], zbp[:])
                    # res.T chunk: catr rows 64r..64r+64 of pair hp
                    avb = work.tile([64, 512], BF16, tag="avb")
                    nc.scalar.copy(avb[:], av[0:DH, :])
                    nc.gpsimd.tensor_mul(
                        catr[64 * r:64 * (r + 1), hp * S + qc * 512:hp * S + (qc + 1) * 512],
                        avb[:], zbs[r][0:64, qc * 512:(qc + 1) * 512])
                # normalized attention out over each k-tile's live window
                for kt in range(KT):
                    W = QWIN[kt]
                    ao = work.tile([128, 384], BF16, tag="ao", bufs=4,
                                   name=f"ao_{h}_{kt}")
                    eng = nc.gpsimd if kt in (0, 4) else nc.vector
                    eng.tensor_mul(ao[:, 0:W], uhsl(kt, r),
                                   zbs[r][:, QLO[kt]:QLO[kt] + W])
                    nc.sync.dma_start(
                        attn_o[h, kt * 128:(kt + 1) * 128, QLO[kt]:QLO[kt] + W],
                        ao[:, 0:W])

        # ---- final linear (+ residual + bias via Wf_aug) + LayerNorm -----
        inv_d = 1.0 / D
        for qt in range(KT):
            p = ps("st")
            for ct in range(4):
                nc.tensor.matmul(
                    p[:],
                    pk1t[:, PK1_DEC + ct * 1024 + qt * 128:PK1_DEC + ct * 1024 + (qt + 1) * 128],
                    pk3t[:, PK3_WF + ct * 512:PK3_WF + (ct + 1) * 512],
                    start=(ct == 0), stop=False)
            for ct in range(4):
                nc.tensor.matmul(
                    p[:],
                    catr[:, ct * S + qt * 128:ct * S + (qt + 1) * 128],
                    pk3t[:, PK3_WF + (4 + ct) * 512:PK3_WF + (5 + ct) * 512],
                    start=False, stop=False)
            nc.tensor.matmul(p[:], gbt[0:1, 2 * D:2 * D + 128],
                             pk3t[0:1, PK3_BF:PK3_BF + 512],
                             start=False, stop=True)
            # LayerNorm over free axis via fused bn_stats/bn_aggr
            bst = work.tile([128, 6], F32, tag="bst")
            nc.vector.bn_stats(bst[:], p[:])
            mv = work.tile([128, 2], F32, tag="mv")
            nc.vector.bn_aggr(mv[:], bst[:])
            sv = work.tile([128, 1], F32, tag="sv")
            nc.scalar.activation(sv[:], mv[:, 1:2], mybir.ActivationFunctionType.Sqrt,
                                 bias=epst[:])
            rstd = work.tile([128, 1], F32, tag="rstd")
            nc.vector.reciprocal(rstd[:], sv[:])
            mb = work.tile([128, 1], F32, tag="mb")
            nc.vector.tensor_scalar(mb[:], mv[:, 0:1], rstd[:], -1.0,
                                    mybir.AluOpType.mult, mybir.AluOpType.mult)
            xn = work.tile([128, D], BF16, tag="xn")
            nc.scalar.activation(xn[:], p[:], mybir.ActivationFunctionType.Identity,
                                 bias=mb[:], scale=rstd[:])
            xg = work.tile([128, D], BF16, tag="xg")
            nc.vector.tensor_mul(xg[:], xn[:], gb[:])
            t1 = work.tile([128, D], F32, tag="t1")
            nc.vector.tensor_add(t1[:], xg[:], bb[:])
            nc.sync.dma_start(out_ln[qt * 128:(qt + 1) * 128, :], t1[:])

    if split:
        _split_multi_waits(nc)
    return nc


_CACHE = {}


def _get_nc():
    if "nc" not in _CACHE:
        _CACHE["nc"] = build()
    return _CACHE["nc"]


def make_in_maps(memory, decoder_input, mask, query_mask, Wk, Wv, Wq, Wf, bf,
                 gamma, beta, gfac):
    memory = np.asarray(memory, dtype=np.float32)
    decoder_input = np.asarray(decoder_input, dtype=np.float32)
    mask = np.asarray(mask)
    query_mask = np.asarray(query_mask, dtype=np.float32)
    gfac = float(np.asarray(gfac))
    # The banded structure (cutoff |k-q| >= 52) is compiled in and derived
    # from gfac=30: exp(-52^2/30) = e^-90 underflows f32. Guard against a
    # different gfac silently producing wrong (truncated) attention.
    assert abs(gfac - 30.0) < 1e-6, (
        f"kernel compiled for gfac=30 (gaussian band cutoff 52), got {gfac}")

    # gaussian factor exp(-(k-q)^2/gfac), [k, q]
    idx = np.arange(S, dtype=np.float64)
    gauss = np.exp(-((idx[:, None] - idx[None, :]) ** 2) / gfac).astype(np.float32)

    WqT = np.ascontiguousarray(np.asarray(Wq, np.float32).T)   # [din, dout]
    WkT = np.ascontiguousarray(np.asarray(Wk, np.float32).T)
    WvT = np.ascontiguousarray(np.asarray(Wv, np.float32).T)
    WfT = np.ascontiguousarray(np.asarray(Wf, np.float32).T)   # [1024, 512]
    Wf_aug = WfT.copy()
    Wf_aug[0:D, :] += np.eye(D, dtype=np.float32)              # residual fold

    def chunks(a, w):
        # [rows, w] -> [128, (rows/128)*w] laid side by side
        r = a.shape[0]
        return np.concatenate([a[i * 128:(i + 1) * 128, :] for i in range(r // 128)],
                              axis=1)

    in_maps = []
    for b in range(B):
        decT = np.ascontiguousarray(decoder_input[b].T)        # [512, 1024]
        memT = np.ascontiguousarray(memory[b].T)
        pk1 = chunks(decT, S)
        bfrow = np.zeros((128, D), np.float32)
        bfrow[0, :] = np.asarray(bf, np.float32)
        pk2 = np.concatenate(
            [chunks(memT, S), chunks(WkT, D), chunks(WvT, D), chunks(WqT, D)], axis=1)
        pk3 = np.concatenate([chunks(Wf_aug[0:1024], D), bfrow], axis=1)
        Eb = np.where(mask[b].T, np.float32(0), gauss)          # [k, q]
        blocks = []
        for kt in range(KT):
            blocks.append(Eb[kt * 128:(kt + 1) * 128, QLO[kt]:QLO[kt] + QWIN[kt]])
        ep = np.concatenate(blocks, axis=1).astype(ml_dtypes.bfloat16)
        qm = query_mask[b][None, :].astype(np.float32)
        gbr = np.concatenate([np.asarray(gamma, np.float32),
                              np.asarray(beta, np.float32),
                              np.ones(128, np.float32)])[None, :]
        in_maps.append({"pk1": np.ascontiguousarray(pk1),
                        "pk2": np.ascontiguousarray(pk2),
                        "pk3": np.ascontiguousarray(pk3),
                        "ep": np.ascontiguousarray(ep),
                        "qm": qm,
                        "gbr": np.ascontiguousarray(gbr)})
    return in_maps


def postprocess(results):
    out = np.empty((B, S, D), np.float32)
    attn = np.empty((B, H, S, S), np.float32)
    for b in range(B):
        out[b] = results[b]["out_ln"]
        # stored [h, k, q] -> reference wants [h, q, k]
        attn[b] = results[b]["attn_o"].astype(np.float32).transpose(0, 2, 1)
    return out, attn


def kernel(**inputs):
    nc = _get_nc()
    in_maps = make_in_maps(**inputs)
    res = run_bass_kernel_spmd(nc, in_maps, core_ids=list(range(N_CORES)))
    return postprocess(res.results)
